# revision 39
# baseline (speedup 1.0000x reference)
"""Trainium2 kernel for nn_AggrEncoder (segment-max + BN + 1x1 conv + fc).

Sharding: pure data-parallel over batch, 4 rows/core on 8 cores.

Host prep (layout only): per batch row, counting-sorts the valid (masked-in)
time columns by window id, pads each window's group to even length and splits
it across two half-arrays A|B at identical pair-positions, so a tensor_tensor
max (the "fold", 2x DVE throughput in bf16) halves the row to T2 pair-columns
with every window still contiguous. A and B are interleaved in DRAM at chunk
granularity (504 pair-cols for rows 0/3, 1008 for rows 1/2) so each fold
chunk depends on a single DMA transfer. BN+conv+fc fold into one (128->8)
affine W_eff/b_eff. All feature payload ships in bf16 (rel-err budget 2e-2;
measured ~6e-3). Rows whose even-padded length exceeds T2 (impossible for
the reference's ~90% mask, but possible for degenerate masks) fall back to a
host-side reduction for that row only.

Device pipeline (DVE is the bottleneck engine at ~64% busy; the
tile_wait_until stamps pin the scheduler to the software-pipelined order
below — its readiness model otherwise hoists all folds ahead of the scans
and starves DVE; every DVE op's producer sits >= 2 slots back so dependency
latency hides behind the intervening op):
  1. All DMAs issue on the SP sequencer: row 0 in four transfers for a fast
     start, b01 slotted after row 0's second chunk (the early HWDGE
     descriptor-gen chain is saturated — any extra gen before a row transfer
     costs ~625 ns of arrival). W_eff/bias ride a 16-col header inside row
     0's first chunk instead of their own DMA.
  2. Fold on DVE per chunk: G = max(A_c, B_c) (bf16 2x mode).
  3. PE ones-matmul replicates pair-level segment-continuation flags into
     PSUM f32 per scan chunk; the first flag matmul is split 128+376 to
     climb PE's p-state; flag chunks for row r+1 run ahead of row r tails.
  4. DVE tensor_tensor_scan (op0=mult, op1=max) over G, flags read straight
     from PSUM. The host aligns every scan-chunk boundary to a window start
     (ROW_BNDS), so all chunks scan carry-free from initial=0. Flag=0 at
     window starts resets the running max and applies the reference
     max(0, .) clamp.
  5. Tail: W_eff^T @ S per <=504-col PSUM bank; ACT scalar.add evacuates
     each bank (bias add + f32->bf16) as soon as it is ready. Row 3 ends
     with 376+128-col units so the final tail/evac/out chain is short.
  6. out DMAs on SP; row 3's ships as [0:1008] on ACT and [1008:2016] on SP
     after the last evacuation.
Host unshard: picks each window's segment-end pair column; empty windows get
b_eff.
"""

import sys

import numpy as np

for _p in ("/opt/trn_rl_repo",):
    if _p not in sys.path:
        sys.path.insert(0, _p)

import concourse.bass as bass
import concourse.bacc as bacc
import concourse.mybir as mybir
from concourse import bass_utils
from concourse._compat import get_trn_type
from concourse.tile import TileContext

import ml_dtypes

B, T, D, Tu, Dout, M = 32, 4096, 128, 512, 64, 8
NCORES = 8
RPC = B // NCORES  # rows per core
BN_EPS = 1e-5

T2 = 2016          # pair columns per row (folded length)
HDR = 16           # row-0 header columns carrying W_eff + bias
BK = 512           # PSUM bank width in f32 — matmul output limit

# per-row A|B interleave grain (DRAM block width in pair columns)
ROW_GRAIN = (504, 1008, 1008, 504)
# per-row fold/scan chunks (lo, width); each chunk sits inside one block.
# row 3 ends with 376+128 so the final output chain is short.
ROW_CHUNKS = (
    ((0, 504), (504, 504), (1008, 504), (1512, 504)),
    ((0, 1008), (1008, 1008)),
    ((0, 1008), (1008, 1008)),
    ((0, 504), (504, 504), (1008, 504), (1512, 376), (1888, 128)),
)
# per-row tail/evac units (lo, width); 504-wide except row 3's fine ending
ROW_TAILS = (
    ((0, 504), (504, 504), (1008, 504), (1512, 504)),
    ((0, 504), (504, 504), (1008, 504), (1512, 504)),
    ((0, 504), (504, 504), (1008, 504), (1512, 504)),
    ((0, 504), (504, 504), (1008, 504), (1512, 376), (1888, 128)),
)
# host chunk-boundary alignment points per row slot
ROW_BNDS = ((504, 1008, 1512), (1008,), (1008,), (504, 1008, 1512, 1888))

_CACHE = {}


def build_bass():
    nc = bacc.Bacc(get_trn_type() or "TRN2", target_bir_lowering=False)

    # row 0 carries a 16-col header in front of the payload: cols 0:8 =
    # W_eff^T (bf16), cols 8:10 = per-partition f32 bias (bitcast). Shipping
    # it inside the first fsort chunk saves a dedicated const DMA whose
    # HWDGE slot would delay the row-1 transfers.
    fsort = nc.dram_tensor(
        "fsort", [RPC, D, HDR + 2 * T2], mybir.dt.bfloat16, kind="ExternalInput"
    )
    b01 = nc.dram_tensor(
        "b01", [1, RPC * T2], mybir.dt.bfloat16, kind="ExternalInput"
    )
    out = nc.dram_tensor(
        "out", [RPC, M, T2], mybir.dt.bfloat16, kind="ExternalOutput"
    )

    with TileContext(nc) as tc:
        with (
            tc.tile_pool(name="const", bufs=1) as cpool,
            tc.tile_pool(name="gpool", bufs=4) as gpool,
            tc.tile_pool(name="fpool", bufs=2) as fpool,
            tc.tile_pool(name="spool", bufs=2) as spool,
            tc.tile_pool(name="opool", bufs=2) as opool,
            tc.tile_pool(name="pbpool", bufs=2, space="PSUM") as ppb,
            tc.tile_pool(name="psum_o", bufs=2, space="PSUM") as ppo,
        ):
            b01_sb = cpool.tile([1, RPC * T2], mybir.dt.bfloat16, tag="b01")
            ones_sb = cpool.tile([1, 128], mybir.dt.bfloat16, tag="ones")
            nc.vector.memset(ones_sb[:], 1.0)

            # ---- fsort DMAs (SP): row 0 in 4 transfers, rest in 2 ----
            FTs = []
            for r in range(RPC):
                FT = gpool.tile([D, HDR + 2 * T2], mybir.dt.bfloat16, tag="FT",
                                name=f"FT{r}")
                FTs.append(FT)
            Q0 = 2 * T2 // 4
            nc.sync.dma_start(FTs[0][:, 0:HDR + Q0], fsort[0][:, 0:HDR + Q0])
            nc.sync.dma_start(FTs[0][:, HDR + Q0:HDR + 2 * Q0],
                              fsort[0][:, HDR + Q0:HDR + 2 * Q0])
            nc.sync.dma_start(b01_sb[:], b01[:])
            for c in range(2, 4):
                lo = HDR + c * Q0
                nc.sync.dma_start(FTs[0][:, lo:lo + Q0],
                                  fsort[0][:, lo:lo + Q0])
            for r in range(1, RPC):
                nc.sync.dma_start(FTs[r][:, HDR:HDR + T2],
                                  fsort[r][:, HDR:HDR + T2])
                nc.sync.dma_start(FTs[r][:, HDR + T2:HDR + 2 * T2],
                                  fsort[r][:, HDR + T2:HDR + 2 * T2])

            weff_ap = FTs[0][:, 0:8]
            bias_ap = FTs[0][:, 8:10].bitcast(mybir.dt.float32)[0:M]

            # ---- per-row structures ----
            Gs, Ss, outs = [], [], []
            for r in range(RPC):
                G = fpool.tile([D, T2], mybir.dt.bfloat16, tag="G", name=f"G{r}")
                S = spool.tile([D, T2], mybir.dt.bfloat16, tag="S", name=f"S{r}")
                o = opool.tile([M, T2], mybir.dt.bfloat16, tag="osb",
                               name=f"osb{r}")
                Gs.append(G)
                Ss.append(S)
                outs.append(o)
            pbs = {}
            pos = {}

            def fold(r, c):
                lo, w = ROW_CHUNKS[r][c]
                gb = ROW_GRAIN[r]
                bs = (lo // gb) * gb          # containing block start
                off = lo - bs
                a0 = HDR + 2 * bs + off
                nc.vector.tensor_tensor(
                    Gs[r][:, lo:lo + w],
                    FTs[r][:, a0:a0 + w],
                    FTs[r][:, a0 + gb:a0 + gb + w],
                    op=mybir.AluOpType.max)

            def reps(r, c):
                lo, w = ROW_CHUNKS[r][c]
                tw_ = 504 if w <= 504 else 1008
                pb = ppb.tile([128, tw_], mybir.dt.float32, tag=f"pb{tw_}",
                              name=f"pb{r}_{c}")
                pbs[(r, c)] = pb[:, 0:w]
                fsrc, fbase = b01_sb, r * T2 + lo
                pieces = list(range(0, w, BK))
                # warm the PE p-state before the first full-width flag matmul
                first = (r, c) == (0, 0)
                for h0 in pieces:
                    pw = min(BK, w - h0)
                    if first and h0 == 0:
                        nc.tensor.matmul(
                            pb[:, 0:128], ones_sb[:],
                            fsrc[:, fbase:fbase + 128],
                            start=True, stop=True)
                        nc.tensor.matmul(
                            pb[:, 128:pw], ones_sb[:],
                            fsrc[:, fbase + 128:fbase + pw],
                            start=True, stop=True)
                    else:
                        nc.tensor.matmul(
                            pb[:, h0:h0 + pw], ones_sb[:],
                            fsrc[:, fbase + h0:fbase + h0 + pw],
                            start=True, stop=True)

            def scan(r, c):
                lo, w = ROW_CHUNKS[r][c]
                # host aligns chunk boundaries to window starts, so every
                # chunk scans independently from a zero carry
                nc.vector.tensor_tensor_scan(
                    Ss[r][:, lo:lo + w],
                    pbs[(r, c)],
                    Gs[r][:, lo:lo + w],
                    0.0,
                    op0=mybir.AluOpType.mult,
                    op1=mybir.AluOpType.max)

            def tail(r, k):
                lo, w = ROW_TAILS[r][k]
                po = ppo.tile([M, 504], mybir.dt.float32, tag="po",
                              name=f"po{r}_{k}")
                pos[(r, k)] = po
                nc.tensor.matmul(
                    po[:, 0:w], weff_ap, Ss[r][:, lo:lo + w],
                    start=True, stop=True)

            def evac(r, k):
                lo, w = ROW_TAILS[r][k]
                nc.scalar.add(outs[r][:, lo:lo + w],
                              pos[(r, k)][:, 0:w], bias_ap)

            def out_dma(r, lo=None, hi=None, eng=None):
                eng = eng or nc.sync
                if lo is None:
                    eng.dma_start(out[r], outs[r][:])
                else:
                    eng.dma_start(out[r][:, lo:hi], outs[r][:, lo:hi])

            # software-pipelined emission, pinned via tile_wait_until.
            # Scans are carry-free (host-aligned chunk boundaries); DVE order
            # keeps every op's producer >= 2 slots back.
            steps = [
                lambda: fold(0, 0), lambda: reps(0, 0),
                lambda: fold(0, 1), lambda: reps(0, 1),
                lambda: scan(0, 0),
                lambda: fold(0, 2), lambda: reps(0, 2),
                lambda: scan(0, 1), lambda: tail(0, 0), lambda: evac(0, 0),
                lambda: fold(0, 3), lambda: reps(0, 3),
                lambda: scan(0, 2), lambda: tail(0, 1), lambda: evac(0, 1),
                lambda: fold(1, 0), lambda: reps(1, 0),
                lambda: scan(0, 3), lambda: tail(0, 2), lambda: evac(0, 2),
                lambda: fold(1, 1), lambda: reps(1, 1),
                lambda: scan(1, 0),
                lambda: tail(0, 3), lambda: evac(0, 3), lambda: out_dma(0),
                lambda: fold(2, 0), lambda: reps(2, 0),
                lambda: scan(1, 1), lambda: tail(1, 0), lambda: evac(1, 0),
                lambda: fold(2, 1), lambda: reps(2, 1),
                lambda: scan(2, 0), lambda: tail(1, 1), lambda: evac(1, 1),
                lambda: tail(1, 2), lambda: evac(1, 2),
                lambda: fold(3, 0), lambda: reps(3, 0),
                lambda: scan(2, 1),
                lambda: tail(1, 3), lambda: evac(1, 3), lambda: out_dma(1),
                lambda: tail(2, 0), lambda: evac(2, 0),
                lambda: fold(3, 1), lambda: reps(3, 1),
                lambda: scan(3, 0), lambda: tail(2, 1), lambda: evac(2, 1),
                lambda: tail(2, 2), lambda: evac(2, 2),
                lambda: fold(3, 2), lambda: reps(3, 2),
                lambda: scan(3, 1),
                lambda: tail(2, 3), lambda: evac(2, 3), lambda: out_dma(2),
                lambda: tail(3, 0), lambda: evac(3, 0),
                lambda: fold(3, 3), lambda: reps(3, 3),
                lambda: scan(3, 2), lambda: tail(3, 1), lambda: evac(3, 1),
                lambda: fold(3, 4), lambda: reps(3, 4),
                lambda: scan(3, 3), lambda: tail(3, 2), lambda: evac(3, 2),
                lambda: scan(3, 4), lambda: tail(3, 3), lambda: evac(3, 3),
                lambda: tail(3, 4), lambda: evac(3, 4),
                lambda: out_dma(3, 0, 1008, eng=nc.scalar),
                lambda: out_dma(3, 1008, 2016),
            ]
            import os
            force = os.environ.get("KW_FORCE", "1") == "1"
            for i, fn in enumerate(steps):
                with tc.tile_wait_until(i * 0.02, enable=force):
                    fn()

    if not nc.is_finalized():
        nc.finalize()
    return nc


def _host_prep(x, mask, tw_uniq, bn_gamma, bn_beta, bn_mean, bn_var,
               conv_w, conv_b, fc_w, fc_b):
    tw = x[:, :, 0]
    u0 = tw_uniq[:, 0, 0]
    idx = np.clip((tw - u0[:, None]).astype(np.int32), 0, Tu - 1)  # (B, T)
    valid = mask[:, :, 0]
    key = np.where(valid, idx, Tu).astype(np.int64)                # (B, T)

    featsT = np.ascontiguousarray(
        x[:, :, 1:].transpose(0, 2, 1)).astype(ml_dtypes.bfloat16)  # (B, D, T)

    fsort = np.zeros((B, D, HDR + 2 * T2), ml_dtypes.bfloat16)
    b01 = np.ones((B, T2), np.float32)
    epos = np.empty((B, Tu), np.int64)

    rowidx = np.arange(T)
    for b in range(B):
        gr = ROW_GRAIN[b % RPC]
        counts = np.bincount(key[b], minlength=Tu + 1)[:Tu]        # (Tu,)
        h = (counts + 1) // 2
        hrow = int(h.sum())
        if hrow > T2:
            # overflow fallback (not expected for the reference's mask):
            # reduce this row host-side and ship one column per window.
            hidden = np.zeros((D, Tu), np.float32)
            fv = featsT[b].astype(np.float32)
            v = valid[b]
            np.maximum.at(hidden.T, idx[b][v], fv[:, v].T)
            hb = np.maximum(hidden, 0.0).astype(ml_dtypes.bfloat16)
            fsort[b] = 0
            # A-columns of each chunk hold the values; B-columns stay 0
            pc = np.arange(Tu)
            acol = HDR + 2 * gr * (pc // gr) + (pc % gr)
            fsort[b][:, acol] = hb
            b01[b] = 1.0
            b01[b][:Tu] = 0.0
            epos[b] = np.where(counts > 0, np.arange(Tu), -1)
            continue

        order = np.argsort(key[b], kind="stable")                  # (T,)
        skey = key[b][order]
        nvalid = int((skey < Tu).sum())
        order = order[:nvalid]
        skey = skey[:nvalid]

        cstart = np.concatenate([[0], np.cumsum(counts)])          # (Tu+1,)
        rank = rowidx[:nvalid] - cstart[skey]                      # pos in segment
        pstart = np.concatenate([[0], np.cumsum(h)[:-1]])          # (Tu,)
        # shift segments so none straddles a scan-chunk boundary; the scans
        # then run carry-free (initial=0) per chunk
        for nb in ROW_BNDS[b % RPC]:
            strad = (pstart < nb) & (pstart + h > nb) & (h > 0)
            if strad.any():
                s0 = int(np.argmax(strad))
                pstart[s0:] += nb - pstart[s0]
        hrow = int((pstart + h)[counts > 0].max()) if (counts > 0).any() else 0
        if hrow > T2:
            hidden = np.zeros((D, Tu), np.float32)
            fv = featsT[b].astype(np.float32)
            v = valid[b]
            np.maximum.at(hidden.T, idx[b][v], fv[:, v].T)
            hb = np.maximum(hidden, 0.0).astype(ml_dtypes.bfloat16)
            fsort[b] = 0
            pc = np.arange(Tu)
            acol = HDR + 2 * gr * (pc // gr) + (pc % gr)
            fsort[b][:, acol] = hb
            b01[b] = 1.0
            b01[b][:Tu] = 0.0
            epos[b] = np.where(counts > 0, np.arange(Tu), -1)
            continue
        hseg = h[skey]
        in_a = rank < hseg
        paircol = pstart[skey] + np.where(in_a, rank, rank - hseg)
        base = 2 * gr * (paircol // gr) + (paircol % gr)
        col = HDR + np.where(in_a, base, base + gr)

        fsort[b][:, col] = featsT[b][:, order]
        b01[b][pstart] = 0.0
        b01[b][hrow:] = 0.0
        epos[b] = np.where(counts > 0, pstart + h - 1, -1)

    s = (bn_gamma.astype(np.float64)
         / np.sqrt(bn_var.astype(np.float64) + BN_EPS))
    t_aff = bn_beta.astype(np.float64) - bn_mean.astype(np.float64) * s
    wc = fc_w.astype(np.float64) @ conv_w.astype(np.float64)       # (8, 128)
    w_eff = wc * s[None, :]
    b_eff = (fc_w.astype(np.float64)
             @ (conv_w.astype(np.float64) @ t_aff + conv_b.astype(np.float64))
             + fc_b.astype(np.float64)).astype(np.float32)         # (8,)

    # row-0 header: cols 0:8 = W_eff^T (bf16), cols 8:10 = f32 bias bitcast
    bias128 = np.zeros((D,), np.float32)
    bias128[:M] = b_eff
    for b in range(0, B, RPC):
        fsort[b][:, 0:8] = w_eff.T.astype(ml_dtypes.bfloat16)
        fsort[b][:, 8:10] = bias128.reshape(D, 1).view(ml_dtypes.bfloat16)

    return fsort, b01.astype(ml_dtypes.bfloat16), epos, b_eff


def _build_in_maps(fsort, b01):
    in_maps = []
    for c in range(NCORES):
        r0 = c * RPC
        in_maps.append(dict(
            fsort=fsort[r0:r0 + RPC],
            b01=b01[r0:r0 + RPC].reshape(1, RPC * T2),
        ))
    return in_maps


def _unshard(core_outs, epos, b_eff):
    final = np.empty((B, Tu, M), np.float32)
    for c in range(NCORES):
        of = core_outs[c]  # (RPC, M, T2) bf16
        for r in range(RPC):
            b = c * RPC + r
            ep = epos[b]
            cols = of[r][:, np.where(ep >= 0, ep, 0)].T.astype(np.float32)
            final[b] = np.where((ep >= 0)[:, None], cols, b_eff[None, :])
    return final


def kernel(x, mask, tw_uniq, bn_gamma, bn_beta, bn_mean, bn_var,
           conv_w, conv_b, fc_w, fc_b):
    fsort, b01, epos, b_eff = _host_prep(
        x, mask, tw_uniq, bn_gamma, bn_beta, bn_mean, bn_var,
        conv_w, conv_b, fc_w, fc_b)

    if "nc" not in _CACHE:
        _CACHE["nc"] = build_bass()
    nc = _CACHE["nc"]

    in_maps = _build_in_maps(fsort, b01)
    res = bass_utils.run_bass_kernel_spmd(nc, in_maps, list(range(NCORES)))
    core_outs = [res.results[c]["out"] for c in range(NCORES)]
    return _unshard(core_outs, epos, b_eff)
